# revision 25
# baseline (speedup 1.0000x reference)
"""Distributed transformer block (B=2, T=2048, C=1024, H=16) on 8 trn2 cores.

Sharding: heads for attention (2 heads/core); tokens for LN/FFN interleaved
across batches (each core owns 256 tokens of batch A + 256 of batch B) so the
post-attention chain for batch A can overlap batch B's attention.

Collectives (in program order on the single CC engine):
  1. kq AllToAll (merged k+q, fired right after the k/q projection groups)
  2. v  AllToAll (overlaps score matmuls for early k-tiles)
  3. att AllToAll for batch A (fired mid-attention; its Wo/LN2/FFN chain
     overlaps batch-B attention emission)
  4. att AllToAll for batch B (only its wire latency is exposed)

Softmax quirk: normalization over the QUERY axis (axis=2 of bhqk). Scores are
computed in [k, q] layout so the normalization is a free-axis rowsum; the
causal mask (valid iff q >= k) is applied with affine_select after exp on the
diagonal 128-block only; 1/rowsum is folded into v before the AV matmul.

Everything on the PE runs in bf16 (transposes included); exp runs on ACT from
wide (up to 1024-col) PSUM score tiles to amortize fixed costs.
"""

import numpy as np
import ml_dtypes

import concourse.bass as bass
import concourse.mybir as mybir
import concourse.tile as tile
from concourse.bass_utils import run_bass_kernel_spmd
from concourse.masks import make_identity

# problem shapes (hardcoded per harness contract)
B, T, C, H = 2, 2048, 1024, 16
HS = C // H          # 64
EPS = 1e-5
NC_ = 8              # cores
TSH = B * T // NC_   # 512 tokens per core (256 per batch, interleaved)
TPB = TSH // B       # 256 tokens per batch per core
HPC = H // NC_       # 2 heads per core
D2 = HPC * HS        # 128 (2 heads side by side)
P = 128
F32 = mybir.dt.float32
BF16 = mybir.dt.bfloat16
F8 = mybir.dt.float8e4
W8SCALE = 32.0

KT = T // P          # 16 k-tiles per batch
QT = T // 512        # 4 q-blocks of 512 per batch
CO = C // P          # 8 chunks of C


def split_waits(nc, max_waits=1):
    """This container's walrus rejects >1 sem-wait per instruction; move
    excess waits onto preceding same-engine NOPs."""
    n = 0
    for bb in nc.main_func.blocks:
        new_insts = []
        for ins in bb.instructions:
            si = ins.sync_info
            if si is not None and si.on_wait and len(si.on_wait) > max_waits:
                waits = list(si.on_wait)
                keep = waits[:max_waits]
                extra = waits[max_waits:]
                chunks = [extra[i:i + max_waits] for i in range(0, len(extra), max_waits)]
                for ci, chunk in enumerate(chunks):
                    new_insts.append(mybir.InstNoOp(
                        name=f"{ins.name}-waitnop{ci}",
                        engine=ins.engine,
                        sync_info=mybir.SyncInfo(on_wait=list(chunk), on_update=[]),
                        text_hint="split_waits",
                    ))
                si.on_wait = keep
                n += 1
            new_insts.append(ins)
        bb.instructions[:] = new_insts
    return n


def _copy_ps(nc, out, in_, use_act):
    """PSUM -> SBUF copy on DVE or ACT (gpsimd cannot read PSUM)."""
    if use_act:
        nc.scalar.activation(out=out, in_=in_,
                             func=mybir.ActivationFunctionType.Copy)
    else:
        nc.vector.tensor_copy(out=out, in_=in_)


def _ln_apply(nc, pool, x_view, out_view, eps_t, tag):
    """LayerNorm (affine folded into weights): out = (x - m) * rsqrt(var+eps).
    x_view: [128, 1024] f32; out_view: [128, 1024] bf16."""
    stats = pool.tile([P, 2, 6], F32, tag=f"{tag}_stats")
    nc.vector.bn_stats(out=stats[:, 0, :], in_=x_view[:, 0:512])
    nc.vector.bn_stats(out=stats[:, 1, :], in_=x_view[:, 512:1024])
    mv = pool.tile([P, 2], F32, tag=f"{tag}_mv")
    nc.vector.bn_aggr(out=mv, in_=stats)
    # mv[:,0]=mean, mv[:,1]=var -> rstd
    nc.scalar.activation(out=mv[:, 1:2], in_=mv[:, 1:2],
                         func=mybir.ActivationFunctionType.Sqrt,
                         bias=eps_t, scale=1.0)
    nc.vector.reciprocal(out=mv[:, 1:2], in_=mv[:, 1:2])
    nc.vector.tensor_scalar(out=out_view, in0=x_view,
                            scalar1=mv[:, 0:1], scalar2=mv[:, 1:2],
                            op0=mybir.AluOpType.subtract,
                            op1=mybir.AluOpType.mult)


import os
DEBUG = os.environ.get("KDEBUG", "0") == "1"


def build_nc():
    nc = bass.Bass(num_devices=NC_, num_swdge_queues=4)

    # ---- per-core external I/O ----
    x_sh = nc.dram_tensor("x_sh", [TSH, C], F32, kind="ExternalInput")
    wqkv = nc.dram_tensor("wqkv", [C, 3 * C], BF16, kind="ExternalInput")
    bqkv = nc.dram_tensor("bqkv", [3, D2], F32, kind="ExternalInput")
    wo = nc.dram_tensor("wo", [C, C], BF16, kind="ExternalInput")
    bo = nc.dram_tensor("bo", [C], F32, kind="ExternalInput")
    w1 = nc.dram_tensor("w1", [C, C], BF16, kind="ExternalInput")
    bf1 = nc.dram_tensor("bf1", [C], F32, kind="ExternalInput")
    w2 = nc.dram_tensor("w2", [C, C], BF16, kind="ExternalInput")
    bf2 = nc.dram_tensor("bf2", [C], F32, kind="ExternalInput")
    out_sh = nc.dram_tensor("out_sh", [TSH, C], F32, kind="ExternalOutput")
    if DEBUG:
        dbg_hT = nc.dram_tensor("dbg_hT", [P, CO, TSH], BF16, kind="ExternalOutput")
        dbg_kT = nc.dram_tensor("dbg_kT", [P, B * T], BF16, kind="ExternalOutput")
        dbg_qT = nc.dram_tensor("dbg_qT", [P, B * T], BF16, kind="ExternalOutput")
        dbg_vT = nc.dram_tensor("dbg_vT", [P, B * T], BF16, kind="ExternalOutput")
        dbg_att0 = nc.dram_tensor("dbg_att0", [P, T], BF16, kind="ExternalOutput")
        dbg_att1 = nc.dram_tensor("dbg_att1", [P, T], BF16, kind="ExternalOutput")
        dbg_x2 = nc.dram_tensor("dbg_x2", [P, 4, C], F32, kind="ExternalOutput")

    rg = [list(range(NC_))]

    with tile.TileContext(nc) as tc:
        with tc.tile_pool(name="persist", bufs=1) as pp, \
             tc.tile_pool(name="dram", bufs=1, space="DRAM") as dp:

            # ---------- constants / persistent weights ----------
            eps_t = pp.tile([P, 1], F32)
            nc.vector.memset(eps_t, EPS)
            ident_f32 = pp.tile([P, P], F32)
            make_identity(nc, ident_f32)
            ident_bf = pp.tile([P, P], BF16)
            nc.vector.tensor_copy(out=ident_bf, in_=ident_f32)

            bqkv_sb = pp.tile([P, 3], F32)
            nc.sync.dma_start(bqkv_sb, bqkv.rearrange("q d -> d q"))
            bf1_sb = pp.tile([P, CO], F32)
            nc.sync.dma_start(bf1_sb, bf1.rearrange("(o i) -> i o", i=P))
            bo_bc = pp.tile([P, C], F32)
            nc.gpsimd.dma_start(bo_bc, bo[:].partition_broadcast(P))
            bf2_bc = pp.tile([P, C], F32)
            nc.gpsimd.dma_start(bf2_bc, bf2[:].partition_broadcast(P))

            # ti 0,1 = batch A; 2,3 = batch B (separate tiles: precise deps)
            x_ts = [pp.tile([P, C], F32, name=f"x_t{ti}") for ti in range(4)]
            for ti in range(4):
                nc.sync.dma_start(x_ts[ti], x_sh[ti * P:(ti + 1) * P, :])

            # wo preloaded early: consumed right after the first att A2A
            wo_sb = pp.tile([P, CO, C], BF16)

            # ---------- P1: LN1 + transpose own shard (bf16) ----------
            sc_p1 = nc.enter_named_scope("P1_ln1", False)
            with tc.tile_pool(name="p1w", bufs=2) as p1w, \
                 tc.tile_pool(name="wq_pool", bufs=1) as wqp, \
                 tc.tile_pool(name="ps_tr", bufs=4, space="PSUM") as ptr, \
                 tc.tile_pool(name="ps_qkv", bufs=4, space="PSUM") as pq:
                # replicated all-head QKV weights [c_i, c_o, (rank,qkv,d2)]
                wqkv_sb = wqp.tile([P, CO, 3 * C], BF16)
                hT_sb = wqp.tile([P, CO, TSH], BF16)  # [c_i, c_o, t_local]
                for cjh in range(2):
                    nc.sync.dma_start(
                        wqkv_sb[:, cjh * 4:(cjh + 1) * 4, :],
                        wqkv.rearrange("(o i) n -> i o n", i=P)[
                            :, cjh * 4:(cjh + 1) * 4, :])
                nc.sync.dma_start(wo_sb, wo.rearrange("(o i) n -> i o n", i=P))
                for ti in range(4):
                    h_t = p1w.tile([P, C], BF16, tag="h_t", name=f"h_t{ti}")
                    _ln_apply(nc, p1w, x_ts[ti], h_t, eps_t, "ln1")
                    for cj in range(CO):
                        ps = ptr.tile([P, P], BF16, tag="tr")
                        nc.tensor.transpose(
                            ps, h_t[:, cj * P:(cj + 1) * P], ident_bf)
                        _copy_ps(nc, hT_sb[:, cj, ti * P:(ti + 1) * P], ps,
                                 (ti + cj) % 2 == 1)
                nc.leave_named_scope("P1_ln1", sc_p1[0], False)

                # ---------- P2: QKV for ALL heads over OWN tokens ----------
                # k+q groups -> one merged A2A; v group -> second A2A.
                if DEBUG:
                    nc.sync.dma_start(dbg_hT[:], hT_sb)
                sc_p2 = nc.enter_named_scope("P2_qkv", False)
                kq_sh = [wqp.tile([P, NC_, 512], BF16, name=f"kq_sh{g}")
                         for g in range(2)]  # g=0: k, g=1: q
                v_sh = wqp.tile([P, NC_, 512], BF16)
                kq_in = [dp.tile([NC_, P, 512], BF16, name=f"kq_a2a_in{g}")
                         for g in range(2)]
                for gi, qkv in enumerate((1, 0)):  # k then q
                    for r in range(NC_):
                        dt_ = r * 3 + qkv
                        psd = pq.tile([P, TSH], F32, tag="psd")
                        for cj in range(CO):
                            nc.tensor.matmul(
                                psd, wqkv_sb[:, cj, dt_ * P:(dt_ + 1) * P],
                                hT_sb[:, cj, :],
                                start=(cj == 0), stop=(cj == CO - 1))
                        _copy_ps(nc, kq_sh[gi][:, r, :], psd, r % 2 == 1)
                    nc.sync.dma_start(
                        kq_in[gi].rearrange("r p t -> p r t"), kq_sh[gi])
                for r in range(NC_):
                    dt_ = r * 3 + 2
                    psd = pq.tile([P, TSH], F32, tag="psd")
                    for cj in range(CO):
                        nc.tensor.matmul(
                            psd, wqkv_sb[:, cj, dt_ * P:(dt_ + 1) * P],
                            hT_sb[:, cj, :],
                            start=(cj == 0), stop=(cj == CO - 1))
                    _copy_ps(nc, v_sh[:, r, :], psd, r % 2 == 1)
                v_in = dp.tile([NC_, P, 512], BF16, name="v_a2a_in")
                nc.sync.dma_start(v_in.rearrange("r p t -> p r t"), v_sh)
                nc.leave_named_scope("P2_qkv", sc_p2[0], False)

            kq_out = [dp.tile([NC_, P, 512], BF16, name=f"kq_a2a_out{g}")
                      for g in range(2)]
            for g in range(2):
                nc.gpsimd.collective_compute(
                    "AllToAll", mybir.AluOpType.bypass,
                    ins=[kq_in[g].opt()], outs=[kq_out[g].opt()],
                    replica_groups=rg)
            v_out = dp.tile([NC_, P, 512], BF16, name="v_a2a_out")
            nc.gpsimd.collective_compute(
                "AllToAll", mybir.AluOpType.bypass,
                ins=[v_in.opt()], outs=[v_out.opt()], replica_groups=rg)

            # ---------- P3 + P4 shared SBUF ----------
            with tc.tile_pool(name="pqkv", bufs=1) as pqk, \
                 tc.tile_pool(name="pffn", bufs=1) as pf:
                # qT/kT: [d2, (b, src, t)]; v_sb: [k_i, blk=(b,kt), d2]
                qT = pqk.tile([P, B * T], BF16)
                kT = pqk.tile([P, B * T], BF16)
                v_sb = pqk.tile([P, B * KT, D2], BF16)

                # ---------- P3: assemble qT/kT/v from the A2As ----------
                sc_p3 = nc.enter_named_scope("P3_asm", False)
                with tc.tile_pool(name="p3w", bufs=1) as p3w, \
                     tc.tile_pool(name="ps_vtr", bufs=4, space="PSUM") as pv:
                    k_v = kq_out[0].rearrange("s p (b t) -> b p s t", b=2)
                    q_v = kq_out[1].rearrange("s p (b t) -> b p s t", b=2)
                    for bb in range(B):
                        nc.sync.dma_start(
                            kT[:, bb * T:(bb + 1) * T].rearrange(
                                "p (s t) -> p s t", s=NC_), k_v[bb])
                    for bb in range(B):
                        nc.sync.dma_start(
                            qT[:, bb * T:(bb + 1) * T].rearrange(
                                "p (s t) -> p s t", s=NC_), q_v[bb])
                    for bb in range(B):
                        tsl = slice(bb * T, (bb + 1) * T)
                        nc.vector.tensor_scalar_add(out=kT[:, tsl],
                                                    in0=kT[:, tsl],
                                                    scalar1=bqkv_sb[:, 1:2])
                        nc.vector.tensor_scalar_add(out=qT[:, tsl],
                                                    in0=qT[:, tsl],
                                                    scalar1=bqkv_sb[:, 0:1])
                    vT_t = p3w.tile([P, B * T], BF16, tag="vT_t")
                    v_v = v_out.rearrange("s p (b t) -> b p s t", b=2)
                    for bb in range(B):
                        nc.sync.dma_start(
                            vT_t[:, bb * T:(bb + 1) * T].rearrange(
                                "p (s t) -> p s t", s=NC_), v_v[bb])
                    nc.vector.tensor_scalar_add(out=vT_t, in0=vT_t,
                                                scalar1=bqkv_sb[:, 2:3])
                    for blk in range(B * KT):
                        ps = pv.tile([P, P], BF16, tag="vtr")
                        nc.tensor.transpose(
                            ps, vT_t[:, blk * P:(blk + 1) * P], ident_bf)
                        _copy_ps(nc, v_sb[:, blk, :], ps, blk % 2 == 1)
                    if DEBUG:
                        nc.sync.dma_start(dbg_kT[:], kT)
                        nc.sync.dma_start(dbg_qT[:], qT)
                        nc.sync.dma_start(dbg_vT[:], vT_t)
                nc.leave_named_scope("P3_asm", sc_p3[0], False)

                # FFN weights: DMA lands during attention
                w1_sb = pf.tile([P, CO, C], BF16)
                nc.sync.dma_start(w1_sb, w1.rearrange("(o i) n -> i o n", i=P))
                w2_sb = pf.tile([P, CO, C], BF16)
                nc.sync.dma_start(w2_sb, w2.rearrange("(o i) n -> i o n", i=P))

                att_outs = []
                # ---------- P4: attention + woven tail ----------
                # Deferred AV: each iteration's AV matmuls are emitted during
                # the NEXT iteration so the PE fills exp-wait stalls. During
                # batch B's kt>=8 region the pair1 score pool is closed and
                # its 2 PSUM banks host the batch-A Wo/LN2/FFN chain, which
                # is woven into the emission stream.
                sc_p4 = nc.enter_named_scope("P4_attn", False)
                with tc.tile_pool(name="p4w", bufs=4) as p4w, \
                     tc.tile_pool(name="ptl", bufs=1) as ptl, \
                     tc.tile_pool(name="ps_att", bufs=1, space="PSUM") as pa:

                    pending = []  # per-iteration AV emitter lists (depth 2)

                    def emit_iter(b, kt, h2, att_ps, sc_alloc):
                        k0 = kt * P
                        jmin = k0 // 512
                        hsl = slice(h2 * HS, (h2 + 1) * HS)
                        wTe = p4w.tile([P, T], BF16, tag="wTe", name="wTe", bufs=6)
                        s_part = p4w.tile([P, 4], F32, tag="s_part", name="s_part")
                        rs = p4w.tile([P, 1], F32, tag="rs", name="rs")
                        # score tiles: dict base -> tile
                        sc_tiles = sc_alloc(kt, h2)
                        for base, sc_t in sc_tiles.items():
                            c0 = max(k0, base)
                            for half in range(2):
                                h0 = base + half * 512
                                h1 = h0 + 512
                                m0 = max(c0, h0)
                                if m0 >= h1:
                                    continue
                                nc.tensor.matmul(
                                    sc_t[:, m0 - base:h1 - base],
                                    kT[hsl, b * T + k0:b * T + k0 + P],
                                    qT[hsl, b * T + m0:b * T + h1],
                                    start=True, stop=True)
                        # older iterations' AV matmuls fill the exp wait
                        while len(pending) >= 2:
                            for av in pending.pop(0):
                                av()
                        # exp segments: diagonal 128-block, then pair-wide
                        edges = [k0, k0 + P]
                        e = (k0 // 1024 + 1) * 1024
                        while e < T + 1:
                            if e > edges[-1]:
                                edges.append(e)
                            e += 1024
                        if edges[-1] != T:
                            edges.append(T)
                        nseg = len(edges) - 1
                        for si in range(nseg):
                            e0, e1 = edges[si], edges[si + 1]
                            base = (e0 // 1024) * 1024
                            sc_t = sc_tiles[base]
                            if si == 0:
                                nc.scalar.activation(
                                    out=wTe[:, e0:e1],
                                    in_=sc_t[:, e0 - base:e1 - base],
                                    func=mybir.ActivationFunctionType.Exp)
                                nc.gpsimd.affine_select(
                                    out=wTe[:, k0:k0 + P],
                                    in_=wTe[:, k0:k0 + P],
                                    compare_op=mybir.AluOpType.is_ge,
                                    fill=0.0, base=0, pattern=[[1, P]],
                                    channel_multiplier=-1)
                                nc.vector.reduce_sum(
                                    out=s_part[:, 0:1], in_=wTe[:, k0:k0 + P],
                                    axis=mybir.AxisListType.X)
                            else:
                                nc.scalar.activation(
                                    out=wTe[:, e0:e1],
                                    in_=sc_t[:, e0 - base:e1 - base],
                                    func=mybir.ActivationFunctionType.Exp)
                                nc.vector.reduce_sum(
                                    out=s_part[:, si:si + 1],
                                    in_=wTe[:, e0:e1],
                                    axis=mybir.AxisListType.X)
                        nc.vector.reduce_sum(out=rs, in_=s_part[:, 0:nseg],
                                             axis=mybir.AxisListType.X)
                        nc.vector.reciprocal(out=rs, in_=rs)
                        cur_avs = []
                        vp = p4w.tile([P, HS], BF16, tag="vp", name="vp", bufs=6)
                        nc.gpsimd.tensor_scalar_mul(
                            out=vp, in0=v_sb[:, b * KT + kt, hsl], scalar1=rs)
                        for j in range(jmin, QT):
                            def av(j=j, kt=kt, k0=k0, h2=h2, vp=vp, wTe=wTe):
                                c0 = max(k0, j * 512)
                                nc.tensor.matmul(
                                    att_ps[j][h2 * HS:(h2 + 1) * HS,
                                              c0 - j * 512:512],
                                    vp, wTe[:, c0:(j + 1) * 512],
                                    start=(kt == 0), stop=(kt == 4 * j + 3),
                                    tile_position=(0, h2 * HS))
                            cur_avs.append(av)
                        pending.append(cur_avs)

                    def finish_batch(b, att_ps):
                        while pending:
                            for av in pending.pop(0):
                                av()
                        attT = ptl.tile([P, T], BF16, tag=f"attT{b}",
                                        name=f"attT{b}")
                        for j in range(QT):
                            nc.scalar.activation(
                                out=attT[:, j * 512:(j + 1) * 512],
                                in_=att_ps[j],
                                func=mybir.ActivationFunctionType.Copy)
                        if DEBUG:
                            nc.sync.dma_start(
                                (dbg_att0 if b == 0 else dbg_att1)[:], attT)
                        a2a_in = dp.tile([NC_, P, TPB], BF16,
                                         name=f"att_a2a_in{b}")
                        nc.sync.dma_start(
                            a2a_in.rearrange("r p t -> p r t"),
                            attT.rearrange("p (r t) -> p r t", r=NC_))
                        a2a_out = dp.tile([NC_, P, TPB], BF16,
                                          name=f"att_a2a_out{b}")
                        nc.gpsimd.collective_compute(
                            "AllToAll", mybir.AluOpType.bypass,
                            ins=[a2a_in.opt()], outs=[a2a_out.opt()],
                            replica_groups=rg)
                        att_outs.append(a2a_out)

                    def make_tail_steps(hb, ptail):
                        """Wo + residual + LN2 + FFN for half hb as a list of
                        small emission steps (weavable)."""
                        attTs = ptl.tile([P, NC_, TPB], BF16, tag=f"attTs{hb}", name=f"attTs{hb}")
                        h2T = ptl.tile([P, CO, TPB], BF16, tag=f"h2T{hb}", name=f"h2T{hb}")
                        uT = ptl.tile([P, CO, TPB], BF16, tag=f"uT{hb}", name=f"uT{hb}")
                        h2_ts = [ptl.tile([P, C], F32, tag=f"h2t{hb}{t2}", name=f"h2t{hb}{t2}")
                                 for t2 in range(2)]
                        steps = []

                        def s_dma():
                            nc.sync.dma_start(
                                attTs,
                                att_outs[hb].rearrange("r d t -> d r t"))
                        steps.append(s_dma)
                        for t2 in range(2):
                            for cj in range(2):
                                def s_wo(t2=t2, cj=cj):
                                    ti = 2 * hb + t2
                                    ps = ptail.tile([P, 512], F32, tag="tail", name="tailps")
                                    for r in range(NC_):
                                        nc.tensor.matmul(
                                            ps,
                                            attTs[:, r, t2 * P:(t2 + 1) * P],
                                            wo_sb[:, r,
                                                  cj * 512:(cj + 1) * 512],
                                            start=(r == 0),
                                            stop=(r == NC_ - 1))
                                    csl = slice(cj * 512, (cj + 1) * 512)
                                    nc.vector.tensor_add(
                                        out=x_ts[ti][:, csl], in0=ps,
                                        in1=x_ts[ti][:, csl])
                                    nc.vector.tensor_add(
                                        out=x_ts[ti][:, csl],
                                        in0=x_ts[ti][:, csl],
                                        in1=bo_bc[:, csl])
                                steps.append(s_wo)
                        for t2 in range(2):
                            def s_ln(t2=t2):
                                _ln_apply(nc, p4w, x_ts[2 * hb + t2],
                                          h2_ts[t2], eps_t, f"ln2_{hb}{t2}")
                            steps.append(s_ln)
                            for cjh in range(2):
                                def s_tr(t2=t2, cjh=cjh):
                                    for cj in range(cjh * 4, cjh * 4 + 4):
                                        ps = ptail.tile([P, 512], F32,
                                                        tag="tail", name="tailps")
                                        nc.tensor.transpose(
                                            ps[:, 0:P],
                                            h2_ts[t2][:, cj * P:(cj + 1) * P],
                                            ident_f32)
                                        nc.vector.tensor_copy(
                                            out=h2T[:, cj,
                                                    t2 * P:(t2 + 1) * P],
                                            in_=ps[:, 0:P])
                                steps.append(s_tr)
                        for jt in range(CO):
                            def s_f1(jt=jt):
                                ps = ptail.tile([P, 512], F32, tag="tail", name="tailps")
                                for cj in range(CO):
                                    nc.tensor.matmul(
                                        ps[:, 0:TPB],
                                        w1_sb[:, cj, jt * P:(jt + 1) * P],
                                        h2T[:, cj, :],
                                        start=(cj == 0), stop=(cj == CO - 1))
                                nc.scalar.activation(
                                    out=uT[:, jt, :], in_=ps[:, 0:TPB],
                                    func=mybir.ActivationFunctionType.Relu,
                                    bias=bf1_sb[:, jt:jt + 1], scale=1.0)
                            steps.append(s_f1)
                        for t2 in range(2):
                            for cj in range(2):
                                def s_f2(t2=t2, cj=cj):
                                    ti = 2 * hb + t2
                                    ps = ptail.tile([P, 512], F32, tag="tail", name="tailps")
                                    for jc in range(CO):
                                        nc.tensor.matmul(
                                            ps,
                                            uT[:, jc, t2 * P:(t2 + 1) * P],
                                            w2_sb[:, jc,
                                                  cj * 512:(cj + 1) * 512],
                                            start=(jc == 0),
                                            stop=(jc == CO - 1))
                                    csl = slice(cj * 512, (cj + 1) * 512)
                                    o_t = p4w.tile([P, 512], F32, tag="o_t", name="o_t")
                                    nc.vector.tensor_add(
                                        out=o_t, in0=ps,
                                        in1=x_ts[ti][:, csl])
                                    nc.vector.tensor_add(
                                        out=o_t, in0=o_t, in1=bf2_bc[:, csl])
                                    nc.sync.dma_start(
                                        out_sh[ti * P:(ti + 1) * P, csl], o_t)
                                steps.append(s_f2)
                        return steps

                    with tc.tile_pool(name="ps_sc0", bufs=1,
                                      space="PSUM") as psc0:
                        def sc_alloc_ab(kt, h2):
                            if kt < 8:
                                return {
                                    0: psc0.tile([P, 1024], F32, tag="sc0", name="sc0"),
                                    1024: psc1.tile([P, 1024], F32,
                                                    tag="sc1", name="sc1"),
                                }
                            pool = psc0 if (kt + h2) % 2 == 0 else psc1
                            tag = "sc0" if (kt + h2) % 2 == 0 else "sc1"
                            return {1024: pool.tile([P, 1024], F32, tag=tag, name=tag)}

                        def sc_alloc_b2(kt, h2):
                            return {1024: psc0.tile([P, 1024], F32,
                                                    tag="sc0", name="sc0")}

                        with tc.tile_pool(name="ps_sc1", bufs=1,
                                          space="PSUM") as psc1:
                            # batch A: full kt loop
                            att_ps = [pa.tile([P, 512], F32, tag=f"att{j}",
                                              name=f"att_ps{j}")
                                      for j in range(QT)]
                            for kt in range(KT):
                                for h2 in range(2):
                                    emit_iter(0, kt, h2, att_ps, sc_alloc_ab)
                            finish_batch(0, att_ps)
                            # batch B: kt 0..7 (needs both pair pools)
                            att_ps_b = [pa.tile([P, 512], F32, tag=f"att{j}",
                                                name=f"att_psB{j}")
                                        for j in range(QT)]
                            for kt in range(8):
                                for h2 in range(2):
                                    emit_iter(1, kt, h2, att_ps_b,
                                              sc_alloc_ab)
                        # psc1 closed: its 2 banks host the tail chain
                        with tc.tile_pool(name="ps_tail", bufs=2,
                                          space="PSUM") as ptail:
                            tail_a = make_tail_steps(0, ptail)
                            ws = 0
                            for kt in range(8, KT):
                                for h2 in range(2):
                                    emit_iter(1, kt, h2, att_ps_b,
                                              sc_alloc_b2)
                                    for _ in range(2):
                                        if ws < len(tail_a):
                                            tail_a[ws]()
                                            ws += 1
                            finish_batch(1, att_ps_b)
                            while ws < len(tail_a):
                                tail_a[ws]()
                                ws += 1
                            # tail for batch B (serial)
                            for step in make_tail_steps(1, ptail):
                                step()
                if DEBUG:
                    for ti in range(4):
                        nc.sync.dma_start(dbg_x2[:][:, ti, :], x_ts[ti])
                nc.leave_named_scope("P4_attn", sc_p4[0], False)

    split_waits(nc)
    return nc


_NC_CACHE = None


def _get_nc():
    global _NC_CACHE
    if _NC_CACHE is None:
        _NC_CACHE = build_nc()
    return _NC_CACHE


def _prep_inputs(inputs):
    """Host-side weight folding + per-core sharding (batch-interleaved)."""
    x = np.asarray(inputs["x"], np.float32)
    Wq, bq = np.asarray(inputs["Wq"], np.float32), np.asarray(inputs["bq"], np.float32)
    Wk, bk = np.asarray(inputs["Wk"], np.float32), np.asarray(inputs["bk"], np.float32)
    Wv, bv = np.asarray(inputs["Wv"], np.float32), np.asarray(inputs["bv"], np.float32)
    Wo, bo = np.asarray(inputs["Wo"], np.float32), np.asarray(inputs["bo"], np.float32)
    g1, b1 = np.asarray(inputs["g1"], np.float32), np.asarray(inputs["b1"], np.float32)
    g2, b2 = np.asarray(inputs["g2"], np.float32), np.asarray(inputs["b2"], np.float32)
    W1, bf1 = np.asarray(inputs["W1"], np.float32), np.asarray(inputs["bf1"], np.float32)
    W2, bf2 = np.asarray(inputs["W2"], np.float32), np.asarray(inputs["bf2"], np.float32)

    scale = float(HS) ** -0.5
    # folded FFN1: h2@W1+bf1 with h2 = ln*g2+b2 -> ln @ (g2*W1) + (b2@W1+bf1)
    w1f = (g2[:, None] * W1).astype(np.float32)
    bf1f = (b2 @ W1 + bf1).astype(np.float32)

    # wqkv_all: [C, 8, 3, 128] ordered (rank, qkv, d2-within-rank), replicated.
    Wq_f = (g1[:, None, None] * Wq.transpose(1, 0, 2).reshape(C, H, HS)
            ).reshape(C, NC_, HPC * HS) * scale
    Wk_f = (g1[:, None, None] * Wk.transpose(1, 0, 2).reshape(C, H, HS)
            ).reshape(C, NC_, HPC * HS)
    Wv_f = (g1[:, None, None] * Wv.transpose(1, 0, 2).reshape(C, H, HS)
            ).reshape(C, NC_, HPC * HS)
    wqkv_all = np.stack([Wq_f, Wk_f, Wv_f], axis=2).reshape(C, 3 * C)
    wqkv_all = np.ascontiguousarray(wqkv_all.astype(ml_dtypes.bfloat16))

    in_maps = []
    for r in range(NC_):
        h0 = HPC * r
        bq_ = (b1 @ Wq[h0:h0 + HPC].transpose(1, 0, 2).reshape(C, D2)
               + bq[h0:h0 + HPC].reshape(D2)) * scale
        bk_ = (b1 @ Wk[h0:h0 + HPC].transpose(1, 0, 2).reshape(C, D2)
               + bk[h0:h0 + HPC].reshape(D2))
        bv_ = (b1 @ Wv[h0:h0 + HPC].transpose(1, 0, 2).reshape(C, D2)
               + bv[h0:h0 + HPC].reshape(D2))
        x_r = np.concatenate([x[0, r * TPB:(r + 1) * TPB],
                              x[1, r * TPB:(r + 1) * TPB]], axis=0)
        in_maps.append({
            "x_sh": np.ascontiguousarray(x_r),
            "wqkv": wqkv_all,
            "bqkv": np.ascontiguousarray(
                np.stack([bq_, bk_, bv_]).astype(np.float32)),
            "wo": np.ascontiguousarray(Wo.astype(ml_dtypes.bfloat16)),
            "bo": np.ascontiguousarray(bo),
            "w1": np.ascontiguousarray(w1f.astype(ml_dtypes.bfloat16)),
            "bf1": np.ascontiguousarray(bf1f),
            "w2": np.ascontiguousarray(W2.astype(ml_dtypes.bfloat16)),
            "bf2": np.ascontiguousarray(bf2),
        })
    return in_maps


def run(inputs, trace=False):
    nc = _get_nc()
    in_maps = _prep_inputs(inputs)
    res = run_bass_kernel_spmd(nc, in_maps, core_ids=list(range(NC_)), trace=trace)
    out = np.empty((B, T, C), np.float32)
    for r in range(NC_):
        o = res.results[r]["out_sh"]
        out[0, r * TPB:(r + 1) * TPB] = o[0:TPB]
        out[1, r * TPB:(r + 1) * TPB] = o[TPB:2 * TPB]
    return out, res


def kernel(**inputs) -> np.ndarray:
    out, _ = run(inputs, trace=False)
    return out


# revision 26
# speedup vs baseline: 1.0243x; 1.0243x over previous
"""Distributed transformer block (B=2, T=2048, C=1024, H=16) on 8 trn2 cores.

Sharding: heads for attention (2 heads/core); tokens for LN/FFN interleaved
across batches (each core owns 256 tokens of batch A + 256 of batch B) so the
post-attention chain for batch A can overlap batch B's attention.

Collectives (in program order on the single CC engine):
  1. kq AllToAll (merged k+q, fired right after the k/q projection groups)
  2. v  AllToAll (overlaps score matmuls for early k-tiles)
  3. att AllToAll for batch A (fired mid-attention; its Wo/LN2/FFN chain
     overlaps batch-B attention emission)
  4. att AllToAll for batch B (only its wire latency is exposed)

Softmax quirk: normalization over the QUERY axis (axis=2 of bhqk). Scores are
computed in [k, q] layout so the normalization is a free-axis rowsum; the
causal mask (valid iff q >= k) is applied with affine_select after exp on the
diagonal 128-block only; 1/rowsum is folded into v before the AV matmul.

Everything on the PE runs in bf16 (transposes included); exp runs on ACT from
wide (up to 1024-col) PSUM score tiles to amortize fixed costs.
"""

import numpy as np
import ml_dtypes

import concourse.bass as bass
import concourse.mybir as mybir
import concourse.tile as tile
from concourse.bass_utils import run_bass_kernel_spmd
from concourse.masks import make_identity

# problem shapes (hardcoded per harness contract)
B, T, C, H = 2, 2048, 1024, 16
HS = C // H          # 64
EPS = 1e-5
NC_ = 8              # cores
TSH = B * T // NC_   # 512 tokens per core (256 per batch, interleaved)
TPB = TSH // B       # 256 tokens per batch per core
HPC = H // NC_       # 2 heads per core
D2 = HPC * HS        # 128 (2 heads side by side)
P = 128
F32 = mybir.dt.float32
BF16 = mybir.dt.bfloat16
F8 = mybir.dt.float8e4
W8SCALE = 32.0

KT = T // P          # 16 k-tiles per batch
QT = T // 512        # 4 q-blocks of 512 per batch
CO = C // P          # 8 chunks of C


def split_waits(nc, max_waits=1):
    """This container's walrus rejects >1 sem-wait per instruction; move
    excess waits onto preceding same-engine NOPs."""
    n = 0
    for bb in nc.main_func.blocks:
        new_insts = []
        for ins in bb.instructions:
            si = ins.sync_info
            if si is not None and si.on_wait and len(si.on_wait) > max_waits:
                waits = list(si.on_wait)
                keep = waits[:max_waits]
                extra = waits[max_waits:]
                chunks = [extra[i:i + max_waits] for i in range(0, len(extra), max_waits)]
                for ci, chunk in enumerate(chunks):
                    new_insts.append(mybir.InstNoOp(
                        name=f"{ins.name}-waitnop{ci}",
                        engine=ins.engine,
                        sync_info=mybir.SyncInfo(on_wait=list(chunk), on_update=[]),
                        text_hint="split_waits",
                    ))
                si.on_wait = keep
                n += 1
            new_insts.append(ins)
        bb.instructions[:] = new_insts
    return n


def _copy_ps(nc, out, in_, use_act):
    """PSUM -> SBUF copy on DVE or ACT (gpsimd cannot read PSUM)."""
    if use_act:
        nc.scalar.activation(out=out, in_=in_,
                             func=mybir.ActivationFunctionType.Copy)
    else:
        nc.vector.tensor_copy(out=out, in_=in_)


def _ln_apply(nc, pool, x_view, out_view, eps_t, tag):
    """LayerNorm (affine folded into weights): out = (x - m) * rsqrt(var+eps).
    x_view: [128, 1024] f32; out_view: [128, 1024] bf16."""
    stats = pool.tile([P, 2, 6], F32, tag=f"{tag}_stats")
    nc.vector.bn_stats(out=stats[:, 0, :], in_=x_view[:, 0:512])
    nc.vector.bn_stats(out=stats[:, 1, :], in_=x_view[:, 512:1024])
    mv = pool.tile([P, 2], F32, tag=f"{tag}_mv")
    nc.vector.bn_aggr(out=mv, in_=stats)
    # mv[:,0]=mean, mv[:,1]=var -> rstd
    nc.scalar.activation(out=mv[:, 1:2], in_=mv[:, 1:2],
                         func=mybir.ActivationFunctionType.Sqrt,
                         bias=eps_t, scale=1.0)
    nc.vector.reciprocal(out=mv[:, 1:2], in_=mv[:, 1:2])
    nc.vector.tensor_scalar(out=out_view, in0=x_view,
                            scalar1=mv[:, 0:1], scalar2=mv[:, 1:2],
                            op0=mybir.AluOpType.subtract,
                            op1=mybir.AluOpType.mult)


import os
DEBUG = os.environ.get("KDEBUG", "0") == "1"


def build_nc():
    nc = bass.Bass(num_devices=NC_, num_swdge_queues=4)

    # ---- per-core external I/O ----
    x_sh = nc.dram_tensor("x_sh", [TSH, C], F32, kind="ExternalInput")
    wqkv = nc.dram_tensor("wqkv", [C, 3 * C], BF16, kind="ExternalInput")
    bqkv = nc.dram_tensor("bqkv", [3, D2], F32, kind="ExternalInput")
    wo = nc.dram_tensor("wo", [C, C], BF16, kind="ExternalInput")
    bo = nc.dram_tensor("bo", [C], F32, kind="ExternalInput")
    w1 = nc.dram_tensor("w1", [C, C], BF16, kind="ExternalInput")
    bf1 = nc.dram_tensor("bf1", [C], F32, kind="ExternalInput")
    w2 = nc.dram_tensor("w2", [C, C], BF16, kind="ExternalInput")
    bf2 = nc.dram_tensor("bf2", [C], F32, kind="ExternalInput")
    out_sh = nc.dram_tensor("out_sh", [TSH, C], F32, kind="ExternalOutput")
    if DEBUG:
        dbg_hT = nc.dram_tensor("dbg_hT", [P, CO, TSH], BF16, kind="ExternalOutput")
        dbg_kT = nc.dram_tensor("dbg_kT", [P, B * T], BF16, kind="ExternalOutput")
        dbg_qT = nc.dram_tensor("dbg_qT", [P, B * T], BF16, kind="ExternalOutput")
        dbg_vT = nc.dram_tensor("dbg_vT", [P, B * T], BF16, kind="ExternalOutput")
        dbg_att0 = nc.dram_tensor("dbg_att0", [P, T], BF16, kind="ExternalOutput")
        dbg_att1 = nc.dram_tensor("dbg_att1", [P, T], BF16, kind="ExternalOutput")
        dbg_x2 = nc.dram_tensor("dbg_x2", [P, 4, C], F32, kind="ExternalOutput")

    rg = [list(range(NC_))]

    with tile.TileContext(nc) as tc:
        with tc.tile_pool(name="persist", bufs=1) as pp, \
             tc.tile_pool(name="dram", bufs=1, space="DRAM") as dp:

            # ---------- constants / persistent weights ----------
            eps_t = pp.tile([P, 1], F32)
            nc.vector.memset(eps_t, EPS)
            ident_f32 = pp.tile([P, P], F32)
            make_identity(nc, ident_f32)
            ident_bf = pp.tile([P, P], BF16)
            nc.vector.tensor_copy(out=ident_bf, in_=ident_f32)

            bqkv_sb = pp.tile([P, 3], F32)
            nc.sync.dma_start(bqkv_sb, bqkv.rearrange("q d -> d q"))
            bf1_sb = pp.tile([P, CO], F32)
            nc.sync.dma_start(bf1_sb, bf1.rearrange("(o i) -> i o", i=P))
            bo_bc = pp.tile([P, C], F32)
            nc.gpsimd.dma_start(bo_bc, bo[:].partition_broadcast(P))
            bf2_bc = pp.tile([P, C], F32)
            nc.gpsimd.dma_start(bf2_bc, bf2[:].partition_broadcast(P))

            # ti 0,1 = batch A; 2,3 = batch B (separate tiles: precise deps)
            x_ts = [pp.tile([P, C], F32, name=f"x_t{ti}") for ti in range(4)]
            for ti in range(4):
                nc.sync.dma_start(x_ts[ti], x_sh[ti * P:(ti + 1) * P, :])

            # wo preloaded early: consumed right after the first att A2A
            wo_sb = pp.tile([P, CO, C], BF16)

            # ---------- P1: LN1 + transpose own shard (bf16) ----------
            sc_p1 = nc.enter_named_scope("P1_ln1", False)
            with tc.tile_pool(name="p1w", bufs=2) as p1w, \
                 tc.tile_pool(name="wq_pool", bufs=1) as wqp, \
                 tc.tile_pool(name="ps_tr", bufs=4, space="PSUM") as ptr, \
                 tc.tile_pool(name="ps_qkv", bufs=4, space="PSUM") as pq:
                # replicated all-head QKV weights [c_i, c_o, (rank,qkv,d2)]
                wqkv_sb = wqp.tile([P, CO, 3 * C], BF16)
                hT_sb = wqp.tile([P, CO, TSH], BF16)  # [c_i, c_o, t_local]
                for cjh in range(2):
                    nc.sync.dma_start(
                        wqkv_sb[:, cjh * 4:(cjh + 1) * 4, :],
                        wqkv.rearrange("(o i) n -> i o n", i=P)[
                            :, cjh * 4:(cjh + 1) * 4, :])
                nc.sync.dma_start(wo_sb, wo.rearrange("(o i) n -> i o n", i=P))
                for ti in range(4):
                    h_t = p1w.tile([P, C], BF16, tag="h_t", name=f"h_t{ti}")
                    _ln_apply(nc, p1w, x_ts[ti], h_t, eps_t, "ln1")
                    for cj in range(CO):
                        ps = ptr.tile([P, P], BF16, tag="tr")
                        nc.tensor.transpose(
                            ps, h_t[:, cj * P:(cj + 1) * P], ident_bf)
                        _copy_ps(nc, hT_sb[:, cj, ti * P:(ti + 1) * P], ps,
                                 (ti + cj) % 2 == 1)
                nc.leave_named_scope("P1_ln1", sc_p1[0], False)

                # ---------- P2: QKV for ALL heads over OWN tokens ----------
                # k+q groups -> one merged A2A; v group -> second A2A.
                if DEBUG:
                    nc.sync.dma_start(dbg_hT[:], hT_sb)
                sc_p2 = nc.enter_named_scope("P2_qkv", False)
                kq_sh = [wqp.tile([P, NC_, 512], BF16, name=f"kq_sh{g}")
                         for g in range(2)]  # g=0: k, g=1: q
                v_sh = wqp.tile([P, NC_, 512], BF16)
                kq_in = [dp.tile([NC_, P, 512], BF16, name=f"kq_a2a_in{g}")
                         for g in range(2)]
                for gi, qkv in enumerate((1, 0)):  # k then q
                    for r in range(NC_):
                        dt_ = r * 3 + qkv
                        psd = pq.tile([P, TSH], F32, tag="psd")
                        for cj in range(CO):
                            nc.tensor.matmul(
                                psd, wqkv_sb[:, cj, dt_ * P:(dt_ + 1) * P],
                                hT_sb[:, cj, :],
                                start=(cj == 0), stop=(cj == CO - 1))
                        _copy_ps(nc, kq_sh[gi][:, r, :], psd, r % 2 == 1)
                    nc.sync.dma_start(
                        kq_in[gi].rearrange("r p t -> p r t"), kq_sh[gi])
                for r in range(NC_):
                    dt_ = r * 3 + 2
                    psd = pq.tile([P, TSH], F32, tag="psd")
                    for cj in range(CO):
                        nc.tensor.matmul(
                            psd, wqkv_sb[:, cj, dt_ * P:(dt_ + 1) * P],
                            hT_sb[:, cj, :],
                            start=(cj == 0), stop=(cj == CO - 1))
                    _copy_ps(nc, v_sh[:, r, :], psd, r % 2 == 1)
                v_in = dp.tile([NC_, P, 512], BF16, name="v_a2a_in")
                nc.sync.dma_start(v_in.rearrange("r p t -> p r t"), v_sh)
                nc.leave_named_scope("P2_qkv", sc_p2[0], False)

            kq_out = [dp.tile([NC_, P, 512], BF16, name=f"kq_a2a_out{g}")
                      for g in range(2)]
            for g in range(2):
                nc.gpsimd.collective_compute(
                    "AllToAll", mybir.AluOpType.bypass,
                    ins=[kq_in[g].opt()], outs=[kq_out[g].opt()],
                    replica_groups=rg)
            v_out = dp.tile([NC_, P, 512], BF16, name="v_a2a_out")
            nc.gpsimd.collective_compute(
                "AllToAll", mybir.AluOpType.bypass,
                ins=[v_in.opt()], outs=[v_out.opt()], replica_groups=rg)

            # ---------- P3 + P4 shared SBUF ----------
            with tc.tile_pool(name="pqkv", bufs=1) as pqk, \
                 tc.tile_pool(name="pffn", bufs=1) as pf:
                # qT/kT: [d2, (b, src, t)]; v_sb: [k_i, blk=(b,kt), d2]
                qT = pqk.tile([P, B * T], BF16)
                kT = pqk.tile([P, B * T], BF16)
                v_sb = pqk.tile([P, B * KT, D2], BF16)

                # ---------- P3: assemble qT/kT/v from the A2As ----------
                sc_p3 = nc.enter_named_scope("P3_asm", False)
                with tc.tile_pool(name="p3w", bufs=1) as p3w, \
                     tc.tile_pool(name="ps_vtr", bufs=4, space="PSUM") as pv:
                    k_v = kq_out[0].rearrange("s p (b t) -> b p s t", b=2)
                    q_v = kq_out[1].rearrange("s p (b t) -> b p s t", b=2)
                    for bb in range(B):
                        nc.sync.dma_start(
                            kT[:, bb * T:(bb + 1) * T].rearrange(
                                "p (s t) -> p s t", s=NC_), k_v[bb])
                    for bb in range(B):
                        nc.sync.dma_start(
                            qT[:, bb * T:(bb + 1) * T].rearrange(
                                "p (s t) -> p s t", s=NC_), q_v[bb])
                    for bb in range(B):
                        tsl = slice(bb * T, (bb + 1) * T)
                        nc.vector.tensor_scalar_add(out=kT[:, tsl],
                                                    in0=kT[:, tsl],
                                                    scalar1=bqkv_sb[:, 1:2])
                        nc.vector.tensor_scalar_add(out=qT[:, tsl],
                                                    in0=qT[:, tsl],
                                                    scalar1=bqkv_sb[:, 0:1])
                    vT_t = p3w.tile([P, B * T], BF16, tag="vT_t")
                    v_v = v_out.rearrange("s p (b t) -> b p s t", b=2)
                    for bb in range(B):
                        nc.sync.dma_start(
                            vT_t[:, bb * T:(bb + 1) * T].rearrange(
                                "p (s t) -> p s t", s=NC_), v_v[bb])
                    nc.vector.tensor_scalar_add(out=vT_t, in0=vT_t,
                                                scalar1=bqkv_sb[:, 2:3])
                    for blk in range(B * KT):
                        ps = pv.tile([P, P], BF16, tag="vtr")
                        nc.tensor.transpose(
                            ps, vT_t[:, blk * P:(blk + 1) * P], ident_bf)
                        _copy_ps(nc, v_sb[:, blk, :], ps, blk % 2 == 1)
                    if DEBUG:
                        nc.sync.dma_start(dbg_kT[:], kT)
                        nc.sync.dma_start(dbg_qT[:], qT)
                        nc.sync.dma_start(dbg_vT[:], vT_t)
                nc.leave_named_scope("P3_asm", sc_p3[0], False)

                # FFN weights: DMA lands during attention
                w1_sb = pf.tile([P, CO, C], BF16)
                nc.sync.dma_start(w1_sb, w1.rearrange("(o i) n -> i o n", i=P))
                w2_sb = pf.tile([P, CO, C], BF16)
                nc.sync.dma_start(w2_sb, w2.rearrange("(o i) n -> i o n", i=P))

                att_outs = []
                # ---------- P4: attention + woven tail ----------
                # Deferred AV: each iteration's AV matmuls are emitted during
                # the NEXT iteration so the PE fills exp-wait stalls. During
                # batch B's kt>=8 region the pair1 score pool is closed and
                # its 2 PSUM banks host the batch-A Wo/LN2/FFN chain, which
                # is woven into the emission stream.
                sc_p4 = nc.enter_named_scope("P4_attn", False)
                with tc.tile_pool(name="p4w", bufs=4) as p4w, \
                     tc.tile_pool(name="ptl", bufs=1) as ptl, \
                     tc.tile_pool(name="ps_att", bufs=1, space="PSUM") as pa:

                    pending = []  # per-iteration AV emitter lists (depth 2)

                    def emit_iter(b, kt, h2, att_ps, sc_alloc):
                        k0 = kt * P
                        jmin = k0 // 512
                        hsl = slice(h2 * HS, (h2 + 1) * HS)
                        wTe = p4w.tile([P, T], BF16, tag="wTe", name="wTe", bufs=6)
                        s_part = p4w.tile([P, 4], F32, tag="s_part", name="s_part")
                        rs = p4w.tile([P, 1], F32, tag="rs", name="rs")
                        # score tiles: dict base -> tile
                        sc_tiles = sc_alloc(kt, h2)
                        for base, sc_t in sc_tiles.items():
                            c0 = max(k0, base)
                            for half in range(2):
                                h0 = base + half * 512
                                h1 = h0 + 512
                                m0 = max(c0, h0)
                                if m0 >= h1:
                                    continue
                                nc.tensor.matmul(
                                    sc_t[:, m0 - base:h1 - base],
                                    kT[hsl, b * T + k0:b * T + k0 + P],
                                    qT[hsl, b * T + m0:b * T + h1],
                                    start=True, stop=True)
                        # older iterations' AV matmuls fill the exp wait
                        while len(pending) >= 2:
                            for av in pending.pop(0):
                                av()
                        # exp segments: diagonal 128-block, then pair-wide
                        edges = [k0, k0 + P]
                        e = (k0 // 1024 + 1) * 1024
                        while e < T + 1:
                            if e > edges[-1]:
                                edges.append(e)
                            e += 1024
                        if edges[-1] != T:
                            edges.append(T)
                        nseg = len(edges) - 1
                        for si in range(nseg):
                            e0, e1 = edges[si], edges[si + 1]
                            base = (e0 // 1024) * 1024
                            sc_t = sc_tiles[base]
                            if si == 0:
                                nc.scalar.activation(
                                    out=wTe[:, e0:e1],
                                    in_=sc_t[:, e0 - base:e1 - base],
                                    func=mybir.ActivationFunctionType.Exp)
                                nc.gpsimd.affine_select(
                                    out=wTe[:, k0:k0 + P],
                                    in_=wTe[:, k0:k0 + P],
                                    compare_op=mybir.AluOpType.is_ge,
                                    fill=0.0, base=0, pattern=[[1, P]],
                                    channel_multiplier=-1)
                                nc.vector.reduce_sum(
                                    out=s_part[:, 0:1], in_=wTe[:, k0:k0 + P],
                                    axis=mybir.AxisListType.X)
                            else:
                                nc.scalar.activation(
                                    out=wTe[:, e0:e1],
                                    in_=sc_t[:, e0 - base:e1 - base],
                                    func=mybir.ActivationFunctionType.Exp)
                                nc.vector.reduce_sum(
                                    out=s_part[:, si:si + 1],
                                    in_=wTe[:, e0:e1],
                                    axis=mybir.AxisListType.X)
                        nc.vector.reduce_sum(out=rs, in_=s_part[:, 0:nseg],
                                             axis=mybir.AxisListType.X)
                        nc.vector.reciprocal(out=rs, in_=rs)
                        cur_avs = []
                        vp = p4w.tile([P, HS], BF16, tag="vp", name="vp", bufs=6)
                        nc.vector.tensor_scalar_mul(
                            out=vp, in0=v_sb[:, b * KT + kt, hsl], scalar1=rs)
                        for j in range(jmin, QT):
                            def av(j=j, kt=kt, k0=k0, h2=h2, vp=vp, wTe=wTe):
                                c0 = max(k0, j * 512)
                                nc.tensor.matmul(
                                    att_ps[j][h2 * HS:(h2 + 1) * HS,
                                              c0 - j * 512:512],
                                    vp, wTe[:, c0:(j + 1) * 512],
                                    start=(kt == 0), stop=(kt == 4 * j + 3),
                                    tile_position=(0, h2 * HS))
                            cur_avs.append(av)
                        pending.append(cur_avs)

                    def finish_batch(b, att_ps):
                        while pending:
                            for av in pending.pop(0):
                                av()
                        attT = ptl.tile([P, T], BF16, tag=f"attT{b}",
                                        name=f"attT{b}")
                        for j in range(QT):
                            nc.scalar.activation(
                                out=attT[:, j * 512:(j + 1) * 512],
                                in_=att_ps[j],
                                func=mybir.ActivationFunctionType.Copy)
                        if DEBUG:
                            nc.sync.dma_start(
                                (dbg_att0 if b == 0 else dbg_att1)[:], attT)
                        a2a_in = dp.tile([NC_, P, TPB], BF16,
                                         name=f"att_a2a_in{b}")
                        nc.sync.dma_start(
                            a2a_in.rearrange("r p t -> p r t"),
                            attT.rearrange("p (r t) -> p r t", r=NC_))
                        a2a_out = dp.tile([NC_, P, TPB], BF16,
                                          name=f"att_a2a_out{b}")
                        nc.gpsimd.collective_compute(
                            "AllToAll", mybir.AluOpType.bypass,
                            ins=[a2a_in.opt()], outs=[a2a_out.opt()],
                            replica_groups=rg)
                        att_outs.append(a2a_out)

                    def make_tail_steps(hb, ptail):
                        """Wo + residual + LN2 + FFN for half hb as a list of
                        small emission steps (weavable)."""
                        attTs = ptl.tile([P, NC_, TPB], BF16, tag=f"attTs{hb}", name=f"attTs{hb}")
                        h2T = ptl.tile([P, CO, TPB], BF16, tag=f"h2T{hb}", name=f"h2T{hb}")
                        uT = ptl.tile([P, CO, TPB], BF16, tag=f"uT{hb}", name=f"uT{hb}")
                        h2_ts = [ptl.tile([P, C], F32, tag=f"h2t{hb}{t2}", name=f"h2t{hb}{t2}")
                                 for t2 in range(2)]
                        steps = []

                        def s_dma():
                            nc.sync.dma_start(
                                attTs,
                                att_outs[hb].rearrange("r d t -> d r t"))
                        steps.append(s_dma)
                        for t2 in range(2):
                            for cj in range(2):
                                def s_wo(t2=t2, cj=cj):
                                    ti = 2 * hb + t2
                                    ps = ptail.tile([P, 512], F32, tag="tail", name="tailps")
                                    for r in range(NC_):
                                        nc.tensor.matmul(
                                            ps,
                                            attTs[:, r, t2 * P:(t2 + 1) * P],
                                            wo_sb[:, r,
                                                  cj * 512:(cj + 1) * 512],
                                            start=(r == 0),
                                            stop=(r == NC_ - 1))
                                    csl = slice(cj * 512, (cj + 1) * 512)
                                    nc.vector.tensor_add(
                                        out=x_ts[ti][:, csl], in0=ps,
                                        in1=x_ts[ti][:, csl])
                                    nc.vector.tensor_add(
                                        out=x_ts[ti][:, csl],
                                        in0=x_ts[ti][:, csl],
                                        in1=bo_bc[:, csl])
                                steps.append(s_wo)
                        for t2 in range(2):
                            def s_ln(t2=t2):
                                _ln_apply(nc, p4w, x_ts[2 * hb + t2],
                                          h2_ts[t2], eps_t, f"ln2_{hb}{t2}")
                            steps.append(s_ln)
                            for cjh in range(2):
                                def s_tr(t2=t2, cjh=cjh):
                                    for cj in range(cjh * 4, cjh * 4 + 4):
                                        ps = ptail.tile([P, 512], F32,
                                                        tag="tail", name="tailps")
                                        nc.tensor.transpose(
                                            ps[:, 0:P],
                                            h2_ts[t2][:, cj * P:(cj + 1) * P],
                                            ident_f32)
                                        nc.vector.tensor_copy(
                                            out=h2T[:, cj,
                                                    t2 * P:(t2 + 1) * P],
                                            in_=ps[:, 0:P])
                                steps.append(s_tr)
                        for jt in range(CO):
                            def s_f1(jt=jt):
                                ps = ptail.tile([P, 512], F32, tag="tail", name="tailps")
                                for cj in range(CO):
                                    nc.tensor.matmul(
                                        ps[:, 0:TPB],
                                        w1_sb[:, cj, jt * P:(jt + 1) * P],
                                        h2T[:, cj, :],
                                        start=(cj == 0), stop=(cj == CO - 1))
                                nc.scalar.activation(
                                    out=uT[:, jt, :], in_=ps[:, 0:TPB],
                                    func=mybir.ActivationFunctionType.Relu,
                                    bias=bf1_sb[:, jt:jt + 1], scale=1.0)
                            steps.append(s_f1)
                        for t2 in range(2):
                            for cj in range(2):
                                def s_f2(t2=t2, cj=cj):
                                    ti = 2 * hb + t2
                                    ps = ptail.tile([P, 512], F32, tag="tail", name="tailps")
                                    for jc in range(CO):
                                        nc.tensor.matmul(
                                            ps,
                                            uT[:, jc, t2 * P:(t2 + 1) * P],
                                            w2_sb[:, jc,
                                                  cj * 512:(cj + 1) * 512],
                                            start=(jc == 0),
                                            stop=(jc == CO - 1))
                                    csl = slice(cj * 512, (cj + 1) * 512)
                                    o_t = p4w.tile([P, 512], F32, tag="o_t", name="o_t")
                                    nc.vector.tensor_add(
                                        out=o_t, in0=ps,
                                        in1=x_ts[ti][:, csl])
                                    nc.vector.tensor_add(
                                        out=o_t, in0=o_t, in1=bf2_bc[:, csl])
                                    nc.sync.dma_start(
                                        out_sh[ti * P:(ti + 1) * P, csl], o_t)
                                steps.append(s_f2)
                        return steps

                    with tc.tile_pool(name="ps_sc0", bufs=1,
                                      space="PSUM") as psc0:
                        def sc_alloc_ab(kt, h2):
                            if kt < 8:
                                return {
                                    0: psc0.tile([P, 1024], F32, tag="sc0", name="sc0"),
                                    1024: psc1.tile([P, 1024], F32,
                                                    tag="sc1", name="sc1"),
                                }
                            pool = psc0 if (kt + h2) % 2 == 0 else psc1
                            tag = "sc0" if (kt + h2) % 2 == 0 else "sc1"
                            return {1024: pool.tile([P, 1024], F32, tag=tag, name=tag)}

                        def sc_alloc_b2(kt, h2):
                            return {1024: psc0.tile([P, 1024], F32,
                                                    tag="sc0", name="sc0")}

                        with tc.tile_pool(name="ps_sc1", bufs=1,
                                          space="PSUM") as psc1:
                            # batch A: full kt loop
                            att_ps = [pa.tile([P, 512], F32, tag=f"att{j}",
                                              name=f"att_ps{j}")
                                      for j in range(QT)]
                            for kt in range(KT):
                                for h2 in range(2):
                                    emit_iter(0, kt, h2, att_ps, sc_alloc_ab)
                            finish_batch(0, att_ps)
                            # batch B: kt 0..7 (needs both pair pools)
                            att_ps_b = [pa.tile([P, 512], F32, tag=f"att{j}",
                                                name=f"att_psB{j}")
                                        for j in range(QT)]
                            for kt in range(8):
                                for h2 in range(2):
                                    emit_iter(1, kt, h2, att_ps_b,
                                              sc_alloc_ab)
                        # psc1 closed: its 2 banks host the tail chain
                        with tc.tile_pool(name="ps_tail", bufs=2,
                                          space="PSUM") as ptail:
                            tail_a = make_tail_steps(0, ptail)
                            ws = 0
                            for kt in range(8, KT):
                                for h2 in range(2):
                                    emit_iter(1, kt, h2, att_ps_b,
                                              sc_alloc_b2)
                                    for _ in range(2):
                                        if ws < len(tail_a):
                                            tail_a[ws]()
                                            ws += 1
                            finish_batch(1, att_ps_b)
                            while ws < len(tail_a):
                                tail_a[ws]()
                                ws += 1
                            # tail for batch B (serial)
                            for step in make_tail_steps(1, ptail):
                                step()
                if DEBUG:
                    for ti in range(4):
                        nc.sync.dma_start(dbg_x2[:][:, ti, :], x_ts[ti])
                nc.leave_named_scope("P4_attn", sc_p4[0], False)

    split_waits(nc)
    return nc


_NC_CACHE = None


def _get_nc():
    global _NC_CACHE
    if _NC_CACHE is None:
        _NC_CACHE = build_nc()
    return _NC_CACHE


def _prep_inputs(inputs):
    """Host-side weight folding + per-core sharding (batch-interleaved)."""
    x = np.asarray(inputs["x"], np.float32)
    Wq, bq = np.asarray(inputs["Wq"], np.float32), np.asarray(inputs["bq"], np.float32)
    Wk, bk = np.asarray(inputs["Wk"], np.float32), np.asarray(inputs["bk"], np.float32)
    Wv, bv = np.asarray(inputs["Wv"], np.float32), np.asarray(inputs["bv"], np.float32)
    Wo, bo = np.asarray(inputs["Wo"], np.float32), np.asarray(inputs["bo"], np.float32)
    g1, b1 = np.asarray(inputs["g1"], np.float32), np.asarray(inputs["b1"], np.float32)
    g2, b2 = np.asarray(inputs["g2"], np.float32), np.asarray(inputs["b2"], np.float32)
    W1, bf1 = np.asarray(inputs["W1"], np.float32), np.asarray(inputs["bf1"], np.float32)
    W2, bf2 = np.asarray(inputs["W2"], np.float32), np.asarray(inputs["bf2"], np.float32)

    scale = float(HS) ** -0.5
    # folded FFN1: h2@W1+bf1 with h2 = ln*g2+b2 -> ln @ (g2*W1) + (b2@W1+bf1)
    w1f = (g2[:, None] * W1).astype(np.float32)
    bf1f = (b2 @ W1 + bf1).astype(np.float32)

    # wqkv_all: [C, 8, 3, 128] ordered (rank, qkv, d2-within-rank), replicated.
    Wq_f = (g1[:, None, None] * Wq.transpose(1, 0, 2).reshape(C, H, HS)
            ).reshape(C, NC_, HPC * HS) * scale
    Wk_f = (g1[:, None, None] * Wk.transpose(1, 0, 2).reshape(C, H, HS)
            ).reshape(C, NC_, HPC * HS)
    Wv_f = (g1[:, None, None] * Wv.transpose(1, 0, 2).reshape(C, H, HS)
            ).reshape(C, NC_, HPC * HS)
    wqkv_all = np.stack([Wq_f, Wk_f, Wv_f], axis=2).reshape(C, 3 * C)
    wqkv_all = np.ascontiguousarray(wqkv_all.astype(ml_dtypes.bfloat16))

    in_maps = []
    for r in range(NC_):
        h0 = HPC * r
        bq_ = (b1 @ Wq[h0:h0 + HPC].transpose(1, 0, 2).reshape(C, D2)
               + bq[h0:h0 + HPC].reshape(D2)) * scale
        bk_ = (b1 @ Wk[h0:h0 + HPC].transpose(1, 0, 2).reshape(C, D2)
               + bk[h0:h0 + HPC].reshape(D2))
        bv_ = (b1 @ Wv[h0:h0 + HPC].transpose(1, 0, 2).reshape(C, D2)
               + bv[h0:h0 + HPC].reshape(D2))
        x_r = np.concatenate([x[0, r * TPB:(r + 1) * TPB],
                              x[1, r * TPB:(r + 1) * TPB]], axis=0)
        in_maps.append({
            "x_sh": np.ascontiguousarray(x_r),
            "wqkv": wqkv_all,
            "bqkv": np.ascontiguousarray(
                np.stack([bq_, bk_, bv_]).astype(np.float32)),
            "wo": np.ascontiguousarray(Wo.astype(ml_dtypes.bfloat16)),
            "bo": np.ascontiguousarray(bo),
            "w1": np.ascontiguousarray(w1f.astype(ml_dtypes.bfloat16)),
            "bf1": np.ascontiguousarray(bf1f),
            "w2": np.ascontiguousarray(W2.astype(ml_dtypes.bfloat16)),
            "bf2": np.ascontiguousarray(bf2),
        })
    return in_maps


def run(inputs, trace=False):
    nc = _get_nc()
    in_maps = _prep_inputs(inputs)
    res = run_bass_kernel_spmd(nc, in_maps, core_ids=list(range(NC_)), trace=trace)
    out = np.empty((B, T, C), np.float32)
    for r in range(NC_):
        o = res.results[r]["out_sh"]
        out[0, r * TPB:(r + 1) * TPB] = o[0:TPB]
        out[1, r * TPB:(r + 1) * TPB] = o[TPB:2 * TPB]
    return out, res


def kernel(**inputs) -> np.ndarray:
    out, _ = run(inputs, trace=False)
    return out
